# revision 11
# baseline (speedup 1.0000x reference)
"""Trainium2 Bass kernel: block 8x8 2D-DCT + channel-pack + 8x nearest upsample.

Computes, for input x (8, 3, 256, 256) f32:
  out[b, 64c+8a+d, 8i+r, 8j+q] = sum_{m,n} D[a,m] x[b,c,8i+m,8j+n] D[d,n]
i.e. the reference nn_DCT2D: per-8x8-block orthonormal DCT-II, 64 coeffs packed
into channels, then 8x8 nearest-neighbor upsample back to (256, 256).

Strategy (pure data-parallel over batch, one core per batch element):
  - Step 1 (TensorE): A2 = X^T @ M'', the row-DCT over H, where M'' is the
    block-diagonal DCT factor with columns permuted to c'' = ie*128+8*ip+a
    (i = 2*ip + ie). Output A2[kh] [128 x 256] for the two w-halves.
  - Step 2 (TensorE): for each row-parity ie, ONE accumulated matmul pair
    psum = sum_kh A2[kh][:, ie-half]^T @ Rc[kh], where Rc folds ALL 8 output
    channel phases d into the columns f = 32*d + j (no upsample in the
    matmul -> 9x less TensorE work than an upsampling rhs).
  - Copy (DVE/ACT): per (d, ie), broadcast-copy psum[:, 32d:32d+32] with a
    double 0-stride AP [128, 8(r), 32(j), 8(q)] into o2 half-tiles
    [128 x 2048], materializing both the 8x H-replication (r) and 8x
    W-replication (q); partition p = 8*ip+a, free f = r*256 + 8*j + q.
  - DMA out (both HWDGE rings): one 2 MB, 128-partition DMA per (c, d)
    with 16 KB descriptors: partition (ip, a) -> channel 64c+8a+d rows
    [16ip, 16ip+16), contiguous in HBM. Tile only has 8 HWDGE completion
    semaphores (max 8 DMAs in flight), so 2 MB per DMA maximizes the
    in-flight byte window (16 MB); 11 slots keep ~3 pre-copied tiles
    ready to issue the moment a semaphore frees.

All consts live in one [128, 1024] tensor (M''|Rc) loaded by a single fast
HWDGE DMA at startup. Everything is f32; matmul accumulation in PSUM f32.
"""

import numpy as np

import concourse.bacc as bacc
import concourse.mybir as mybir
from concourse.tile import TileContext
from concourse.bass_utils import run_bass_kernel_spmd

N_CORES = 8
B, C, H, W = 8, 3, 256, 256
BS = 8          # DCT block size
F32 = mybir.dt.float32


def _dct_matrix() -> np.ndarray:
    n = np.arange(BS, dtype=np.float64)
    k = n[:, None]
    D = np.cos(np.pi * (2.0 * n[None, :] + 1.0) * k / (2.0 * BS))
    scale = np.full((BS,), np.sqrt(2.0 / BS))
    scale[0] = np.sqrt(1.0 / BS)
    return (D * scale[:, None]).astype(np.float32)


def _build_consts() -> np.ndarray:
    D = _dct_matrix()
    # consts [128, 1024]: cols [kt*256 + c''] = M''[kt], cols [512 + kh*256 + f] = Rc[kh]
    consts = np.zeros((128, 1024), np.float32)
    # M'' [2, 128, 256]: col c'' = ie*128 + 8*ip + a maps to DCT row
    # 8a + (k%8)... : M''[kt][p, c''] = D[a, k%8] with k = kt*128+p, i = k//8,
    # iff c'' == (i%2)*128 + 8*(i//2 % 16) + a.
    for k in range(256):
        i = k // 8
        for a in range(8):
            cpp = (i % 2) * 128 + 8 * ((i // 2) % 16) + a
            consts[k % 128, (k // 128) * 256 + cpp] = D[a, k % 8]
    # Rc [2, 128, 256]: Rc[kh][k', 32d + j] = D[d, k'%8] iff j == k'//8 + 16*kh.
    for kh in range(2):
        for kp in range(128):
            j = kp // 8 + 16 * kh
            for d in range(8):
                consts[kp, 512 + kh * 256 + 32 * d + j] = D[d, kp % 8]
    return consts


def _build_module():
    nc = bacc.Bacc("TRN2", target_bir_lowering=False, debug=False,
                   enable_asserts=False)

    x_t = nc.dram_tensor("x", [C, H, W], F32, kind="ExternalInput")
    c_t = nc.dram_tensor("consts", [128, 1024], F32, kind="ExternalInput")
    out_t = nc.dram_tensor("out", [C * 64, H, W], F32, kind="ExternalOutput")
    # view for full-tile stores: [c, d, ip, a, (hh w)]
    out_r = out_t.rearrange(
        "(c a d) (ip hh) w -> c d ip a (hh w)", c=C, a=8, d=8, ip=16)

    with TileContext(nc) as tc:
        with (
            tc.tile_pool(name="consts", bufs=1) as cpool,
            tc.tile_pool(name="xp", bufs=2) as xpool,
            tc.tile_pool(name="atp", bufs=4) as atpool,
            tc.tile_pool(name="outp", bufs=11) as opool,
            tc.tile_pool(name="psa", bufs=2, space="PSUM") as psa_pool,
            tc.tile_pool(name="ps2", bufs=4, space="PSUM") as ps2_pool,
        ):
            ct = cpool.tile([128, 1024], F32, tag="c")
            nc.sync.dma_start(out=ct[:, :], in_=c_t[:, :])

            for c in range(C):
                # load image c as one [128, 512] tile: f = kt*256 + w
                xt = xpool.tile([128, 512], F32, tag="x")
                nc.gpsimd.dma_start(
                    out=xt[:, :].rearrange("p (kt w) -> p kt w", kt=2),
                    in_=x_t[c].rearrange("(kt p) w -> p kt w", kt=2))

                # step 1: A2[kh] [w-in-kh-half, c''=(ie, ip, a)]
                at = []
                for kh in range(2):
                    ps_a = psa_pool.tile([128, 256], F32, tag="psa")
                    for kt in range(2):
                        nc.tensor.matmul(
                            ps_a[:, :],
                            lhsT=xt[:, kt * 256 + kh * 128:
                                    kt * 256 + kh * 128 + 128],
                            rhs=ct[:, kt * 256:(kt + 1) * 256],
                            start=(kt == 0), stop=(kt == 1),
                        )
                    a_sb = atpool.tile([128, 256], F32, tag="at")
                    nc.vector.tensor_copy(out=a_sb[:, :], in_=ps_a[:, :])
                    at.append(a_sb)

                # step 2: one accumulated matmul pair per ie -> all 8 d's
                ps2 = []
                for ie in range(2):
                    ps = ps2_pool.tile([128, 256], F32, tag="ps2")
                    for kh in range(2):
                        nc.tensor.matmul(
                            ps[:, :],
                            lhsT=at[kh][:, ie * 128:(ie + 1) * 128],
                            rhs=ct[:, 512 + kh * 256:512 + (kh + 1) * 256],
                            start=(kh == 0), stop=(kh == 1),
                        )
                    ps2.append(ps)

                # copies + DMA per d: both upsamples via double-broadcast
                # AP; one 2 MB DMA per tile.
                for d in range(8):
                    o4 = opool.tile([128, 4096], F32, tag="o4")
                    for ie in range(2):
                        src = ps2[ie][:, None, 32 * d:32 * d + 32, None] \
                            .to_broadcast([128, 8, 32, 8])
                        dst = o4[:, ie * 2048:(ie + 1) * 2048].rearrange(
                            "p (r j q) -> p r j q", r=8, j=32)
                        if (d + ie) % 2 == 0:
                            nc.vector.tensor_copy(out=dst, in_=src)
                        else:
                            nc.scalar.copy(out=dst, in_=src)
                    eng = nc.sync if d % 2 == 0 else nc.scalar
                    eng.dma_start(out=out_r[c, d], in_=o4[:, :])

    nc.compile()
    return nc


_CACHE: dict = {}


def _get_module():
    if "nc" not in _CACHE:
        _CACHE["nc"] = _build_module()
        _CACHE["consts"] = _build_consts()
    return _CACHE["nc"], _CACHE["consts"]


def _in_maps(x: np.ndarray):
    _, consts = _get_module()
    return [{"x": x[b], "consts": consts} for b in range(N_CORES)]


def kernel(x: np.ndarray) -> np.ndarray:
    x = np.ascontiguousarray(np.asarray(x, dtype=np.float32))
    assert x.shape == (B, C, H, W), x.shape

    nc, _ = _get_module()
    res = run_bass_kernel_spmd(nc, _in_maps(x), core_ids=list(range(N_CORES)))
    out = np.stack([res.results[b]["out"] for b in range(N_CORES)], axis=0)
    return out
